# revision 23
# baseline (speedup 1.0000x reference)
"""Trainium2 Bass kernel for nn_Detection (conv-embed + shared-head gumbel attention).

Strategy (data-parallel over batch, 8 cores):
  - conv(3x3,6->6) + channel-first flatten + fc1(1014->32) is one linear map of
    the raw [13,13,6] observation; folded on host into Wf [1014->32] (exact).
  - Per core 8192 samples (1024 self + 7168 other), shipped feature-major
    (transposed) in fp8e4m3: score/SCALE is O(3e-5) against O(1) gumbel noise,
    so fp8 inputs give ~3e-5 final rel err (validated vs fp32 reference).
  - score(bt,a) = q(bt).e2o(bt,a), q = (A^T e2s + u)/SCALE, A = key_w^T sel_w.
    The v.e2s + key_b.sel_b terms are constant over a => softmax-invariant,
    dropped exactly.
  - All embed matmuls have M=32; four sample-tiles are stacked per PSUM bank
    via tile_position col/row tiling, so leaky/elementwise ops run 4-wide and
    the score partition-reduction is a single K=128 0/1-matmul per column.
  - Softmax denominator / head-mean / reward reductions: small fp32 matmuls.
  - Two bt-halves pipeline against the (k, half)-ordered X DMA stream.

Walrus on this toolchain allows ONE sync wait per compute instruction, so:
  - "claim" dummy matmuls take the PSUM bank-reuse drain wait,
  - 1-element ldweights ("witness" reads) pre-consume cross-engine slot
    releases on PE,
  - 1-element DVE copies pre-consume cross-engine/DMA deps on DVE,
  - output DMAs ride SWDGE (gpsimd).

Per-core sample column order: [h0: self | a0..a6 | h1: self | a0..a6] x 512.
Stack columns (PSUM partition slots s=0..3, E2/P layout):
  c=0: [a0,a1,a2,a3]@h0   c=1: [self,a4,a5,a6]@h0   c=2,3: same @h1
"""
import numpy as np
import ml_dtypes

B, T, A1 = 32, 256, 7
HEADS = 4
NCORES = 8
BPC = B // NCORES            # batches per core
BT = BPC * T                 # 1024 (b,t) pairs per core
NOTH = BT * A1               # 7168 other samples
NS = BT + NOTH               # 8192 samples per core
IN_FEAT = 169 * 6            # 1014
KF = 1024                    # padded feature dim
KCH = KF // 128              # 8 feature chunks
SCALE = 16.0 * float(np.sqrt(A1 * B * T * 32))
F8 = ml_dtypes.float8_e4m3
BF16 = ml_dtypes.bfloat16

# stack-column layout: COLS[cl] = tile content per slot ('self' or a index);
# half h uses columns 2h (cl=0) and 2h+1 (cl=1).
COLS = [[0, 1, 2, 3], ['self', 4, 5, 6]]

_NC_CACHE = {}


def _fold_conv_fc1(conv_w, conv_b, fc1_w, fc1_b):
    """Fold conv+flatten+fc1 into Wf[32, 1014] acting on raw NHWC-flat input."""
    f1 = fc1_w.reshape(32, 6, 13, 13).astype(np.float64)
    Wf = np.zeros((32, 13, 13, 6), np.float64)
    for kh in range(3):
        for kw in range(3):
            i0, i1 = max(0, 1 - kh), min(13, 14 - kh)
            j0, j1 = max(0, 1 - kw), min(13, 14 - kw)
            contrib = np.einsum("oaij,ac->oijc", f1[:, :, i0:i1, j0:j1],
                                conv_w[:, :, kh, kw].astype(np.float64))
            Wf[:, i0 + kh - 1:i1 + kh - 1, j0 + kw - 1:j1 + kw - 1, :] += contrib
    bf = fc1_b.astype(np.float64) + np.einsum("ocij,c->o", f1, conv_b.astype(np.float64))
    return Wf.reshape(32, IN_FEAT).astype(np.float32), bf.astype(np.float32)


def _build_nc(zero_b1=True, zero_b2=True, zero_u=True):
    import concourse.bass as bass
    import concourse.tile as tile
    import concourse.mybir as mybir
    from contextlib import ExitStack
    dt = mybir.dt
    f32, bf16, f8 = dt.float32, dt.bfloat16, dt.float8e4

    nc = bass.Bass()
    xt = nc.declare_dram_parameter("xt", [KCH, 128, NS], f8, isOutput=False)
    cblob = nc.declare_dram_parameter("cblob", [128, 2240], f32, isOutput=False)
    outc = nc.declare_dram_parameter("outc", [8, BT], f32, isOutput=True)

    MULT, MAX, ADD = mybir.AluOpType.mult, mybir.AluOpType.max, mybir.AluOpType.add
    EXP = mybir.ActivationFunctionType.Exp

    def leaky_pair(ve, ps, bias_ap, tmp, out_ap):
        """out = leaky(ps + b) = max(0.01*(ps+b), ps+b), PSUM -> SBUF.

        bias_ap is None when the folded bias is exactly zero (true for this
        model's setup_inputs); the nonzero-bias form uses per-partition APs.
        """
        if bias_ap is None:
            ve.tensor_scalar(out=tmp[:], in0=ps[:], scalar1=0.01, scalar2=None,
                             op0=MULT)
            ve.scalar_tensor_tensor(out=out_ap, in0=ps[:], scalar=1.0,
                                    in1=tmp[:], op0=MULT, op1=MAX)
        else:
            ve.tensor_scalar(out=tmp[:], in0=ps[:], scalar1=bias_ap, scalar2=0.01,
                             op0=ADD, op1=MULT)
            ve.scalar_tensor_tensor(out=out_ap, in0=ps[:], scalar=bias_ap,
                                    in1=tmp[:], op0=ADD, op1=MAX)

    with tile.TileContext(nc) as tc, ExitStack() as ctx:
        const = ctx.enter_context(tc.tile_pool(name="const", bufs=1))
        xp = ctx.enter_context(tc.tile_pool(name="xp", bufs=1))
        actp = ctx.enter_context(tc.tile_pool(name="actp", bufs=1))
        work = ctx.enter_context(tc.tile_pool(name="work", bufs=3))
        outp = ctx.enter_context(tc.tile_pool(name="outp", bufs=1))
        ps_fcb = ctx.enter_context(tc.tile_pool(name="ps_fcb", bufs=4, space="PSUM"))
        ps_qp = ctx.enter_context(tc.tile_pool(name="ps_qp", bufs=1, space="PSUM"))
        ps_scp = ctx.enter_context(tc.tile_pool(name="ps_scp", bufs=1, space="PSUM"))
        ps_sm = ctx.enter_context(tc.tile_pool(name="ps_sm", bufs=2, space="PSUM"))

        # ---- single const blob (f32 carrier, bf16/fp8 views via bitcast) ----
        sb_cb = const.tile([128, 2240], f32)
        nc.sync.dma_start(out=sb_cb[:], in_=cblob[:])
        sb_gum = sb_cb[0:28, 0:1024]
        sb_diff = sb_cb[0:7, 1024:2048]
        sb_bones = sb_cb[0:28, 2048:2076]
        sb_hmean = sb_cb[0:28, 2076:2083]
        sb_ones7 = sb_cb[0:7, 2083:2084]
        sb_b1s = sb_cb[:, 2084:2085]
        sb_b2s = sb_cb[:, 2085:2086]
        sb_us = sb_cb[:, 2086:2087]
        sb_fc2ts = sb_cb[:, 2088:2104].bitcast(bf16)
        sb_smat4 = sb_cb[:, 2104:2160].bitcast(bf16)
        sb_aext = sb_cb[0:32, 2160:2176].bitcast(bf16)
        wfall = sb_cb[:, 2176:2240].bitcast(f8)
        sb_wf = [wfall[:, 32 * k:32 * k + 32] for k in range(KCH)]

        # ---- X: one DMA per bt-half (keeps HWDGE lane count at 3) ----
        sb_xt = xp.tile([128, KCH * NS], f8)
        sb_x = [sb_xt[:, k * NS:(k + 1) * NS] for k in range(KCH)]
        xv = sb_xt[:].rearrange("p (k s) -> p k s", k=KCH)
        for h in range(2):
            hs = h * 4096
            nc.sync.dma_start(out=xv[:, :, hs:hs + 4096],
                              in_=xt[:, :, hs:hs + 4096].rearrange("k p s -> p k s"))

        sb_scr = actp.tile([1, 64], f32)
        sb_e1 = actp.tile([128, 2048], bf16)   # E1 tile nt at [32*(nt%4), 512*(nt//4)]
        sb_e2 = actp.tile([128, 2048], bf16)   # E2 by stack-column layout
        sb_p = actp.tile([128, 2048], bf16)

        sb_logits = actp.tile([28, BT], f32)
        sb_exp = actp.tile([28, BT], f32)
        sb_norm = actp.tile([28, BT], f32)
        sb_tout = outp.tile([7, BT], f32)
        sb_rout = outp.tile([1, BT], f32)

        scratch = {"n": 0}

        def pre(eng, *aps):
            # 1-element copies that pre-consume cross-engine/DMA dependencies
            # on this engine, so the following real op carries at most one
            # sync wait (walrus allows a single wait per compute instruction).
            for ap in aps:
                if ap is None:
                    continue
                col = scratch["n"] % 64
                scratch["n"] += 1
                eng.tensor_copy(out=sb_scr[0:1, col:col + 1], in_=ap)

        def claim(ps, after=()):
            # Tiny dummy matmul claiming a reused PSUM bank: it carries the
            # BankOverlapTracker's PE-drain wait so the real (data-waiting)
            # matmuls that follow need no second sync wait slot.  Nosync
            # edges pin it after its witness loads in the schedule.
            mm = nc.tensor.matmul(ps[0:1, 0:1], sb_wf[0][:, 0:1],
                                  sb_wf[0][:, 0:1], start=True, stop=True,
                                  skip_group_check=True)
            for wi in after:
                bass._add_dep_helper(mm.ins, wi.ins, sync=False,
                                     reason="claim after witness")
            return mm

        def witness(*aps):
            # 1-element ldweights that pre-consume cross-engine slot-release
            # deps on the PE before a reused PSUM tile's first matmul.
            out = []
            for ap in aps:
                if ap is None:
                    continue
                if ap.dtype == f32:
                    ap = ap.bitcast(bf16)
                out.append(nc.tensor.ldweights(ap[0:1, 0:1]))
            return out

        # per-pool history of release-witness APs, for slot-reuse absorption
        hist = {"fcb": [], "qp": [], "scp": [], "sm": []}
        BUFS = {"fcb": 4, "qp": 1, "scp": 1, "sm": 2}

        def acquire(pool, new_tile_cb, extra=()):
            h_ = hist[pool]
            reused = len(h_) >= BUFS[pool]
            wits = []
            if reused:
                wits = witness(*h_[len(h_) - BUFS[pool]], *extra)
            h_.append([])
            tile_ = new_tile_cb()
            if reused:
                claim(tile_, after=wits)
            return h_[-1], tile_

        for h in range(2):
            hcols = slice(h * 512, h * 512 + 512)
            # ---- fc1 for this half: 8 sample tiles in 2 stacked banks ----
            banks, bank_wit = [], []
            for gl in range(2):
                g = 2 * h + gl
                wit, ps = acquire("fcb", lambda: ps_fcb.tile(
                    [128, 512], f32, name=f"fc1b{g}", tag="fcb"))
                banks.append(ps)
                bank_wit.append(wit)
            for k in range(KCH):
                for gl in range(2):
                    g = 2 * h + gl
                    for s in range(4):
                        nt = 4 * g + s
                        c0 = nt * 512
                        nc.tensor.matmul(banks[gl][32 * s:32 * s + 32, :],
                                         sb_wf[k][:], sb_x[k][:, c0:c0 + 512],
                                         start=(k == 0), stop=(k == KCH - 1),
                                         tile_position=(0, 32 * s),
                                         skip_group_check=True)
            for gl in range(2):
                g = 2 * h + gl
                tmp = work.tile([128, 512], f32, tag="lk", bufs=8)
                leaky_pair(nc.vector, banks[gl],
                           None if zero_b1 else sb_b1s[:, 0:1], tmp,
                           sb_e1[:, g * 512:(g + 1) * 512])
                bank_wit[gl].append(sb_e1[0:1, g * 512:g * 512 + 1])

            # ---- fc2 into stack-column layout + leaky ----
            for cl in range(2):
                c = 2 * h + cl
                cwit, psc = acquire("fcb", lambda: ps_fcb.tile(
                    [128, 512], f32, name=f"fc2c{c}", tag="fcb"))
                for s, content in enumerate(COLS[cl]):
                    idx = 0 if content == 'self' else 1 + content
                    nt = 8 * h + idx
                    r, g1 = nt % 4, nt // 4
                    nc.tensor.matmul(
                        psc[32 * s:32 * s + 32, :],
                        sb_fc2ts[32 * r:32 * r + 32, :],
                        sb_e1[32 * r:32 * r + 32, g1 * 512:(g1 + 1) * 512],
                        tile_position=(32 * r, 32 * s), skip_group_check=True)
                tmp = work.tile([128, 512], f32, tag="lk", bufs=8)
                leaky_pair(nc.vector, psc,
                           None if zero_b2 else sb_b2s[:, 0:1], tmp,
                           sb_e2[:, c * 512:(c + 1) * 512])
                cwit.append(sb_e2[0:1, c * 512:c * 512 + 1])

            # ---- q (4-stacked): q = aext^T . e2self ----
            qwit, psq = acquire("qp", lambda: ps_qp.tile(
                [128, 512], f32, name=f"q{h}", tag="q"))
            selfc = (2 * h + 1) * 512            # E2 column of the self tile, slot 0
            for s in range(4):
                nc.tensor.matmul(psq[32 * s:32 * s + 32, :], sb_aext[:],
                                 sb_e2[0:32, selfc:selfc + 512],
                                 tile_position=(0, 32 * s),
                                 skip_group_check=True)

            # ---- P = (q + u) * e2o ; score = smat4^T . P  (K=128) ----
            scwit, pssc = acquire("scp", lambda: ps_scp.tile(
                [28, 512], f32, name=f"sc{h}", tag="sc"))
            for cl in range(2):
                c = 2 * h + cl
                ccols = slice(c * 512, (c + 1) * 512)
                if cl == 0:
                    pre(nc.vector, psq[0:1, 0:1])
                if zero_u:
                    nc.vector.tensor_mul(sb_p[:, ccols], psq[:], sb_e2[:, ccols])
                else:
                    nc.vector.scalar_tensor_tensor(
                        out=sb_p[:, ccols], in0=psq[:], scalar=sb_us[:, 0:1],
                        in1=sb_e2[:, ccols], op0=ADD, op1=MULT)
                nc.tensor.matmul(pssc[:], sb_smat4[:, 28 * c:28 * c + 28],
                                 sb_p[:, ccols], start=(cl == 0), stop=(cl == 1))
            qwit.append(sb_p[0:1, (2 * h + 1) * 512:(2 * h + 1) * 512 + 1])

            # ---- softmax over a, mean over heads, reward ----
            pre(nc.vector, pssc[0:1, 0:1], sb_gum[0:1, h * 512:h * 512 + 1])
            nc.vector.tensor_add(sb_logits[:, hcols], pssc[:], sb_gum[:, hcols])
            scwit.append(sb_logits[0:1, h * 512:h * 512 + 1])
            nc.scalar.activation(sb_exp[:, hcols], sb_logits[:, hcols], EXP)
            dwit, den = acquire("sm", lambda: ps_sm.tile(
                [28, 512], f32, name=f"den{h}", tag="sm"))
            nc.tensor.matmul(den[:], sb_bones[:], sb_exp[:, hcols])
            rd = work.tile([28, 512], f32, tag="rd")
            nc.vector.reciprocal(rd[:], den[:])
            pre(nc.vector, sb_exp[0:1, h * 512:h * 512 + 1])
            nc.vector.tensor_mul(sb_norm[:, hcols], sb_exp[:, hcols], rd[:])
            dwit.append(sb_norm[0:1, h * 512:h * 512 + 1])
            twit, th = acquire("sm", lambda: ps_sm.tile(
                [7, 512], f32, name=f"th{h}", tag="sm"))
            nc.tensor.matmul(th[:], sb_hmean[:], sb_norm[:, hcols])
            nc.scalar.copy(out=sb_tout[:, hcols], in_=th[:])
            td = work.tile([7, 512], f32, tag="td")
            pre(nc.vector, th[0:1, 0:1], sb_diff[0:1, 0:1])
            nc.vector.tensor_mul(td[:], th[:], sb_diff[:, hcols])
            twit.append(sb_tout[0:1, h * 512:h * 512 + 1])
            twit.append(td[0:1, 0:1])
            rwit, rw = acquire("sm", lambda: ps_sm.tile(
                [1, 512], f32, name=f"rw{h}", tag="sm"), extra=(td[0:1, 0:1],))
            nc.tensor.matmul(rw[:], sb_ones7[:], td[:])
            nc.scalar.copy(out=sb_rout[:, hcols], in_=rw[:])
            rwit.append(sb_rout[0:1, h * 512:h * 512 + 1])
        nc.gpsimd.dma_start(out=outc[0:7, :], in_=sb_tout[:])
        nc.gpsimd.dma_start(out=outc[7:8, :], in_=sb_rout[:])

    return nc


def _split_drain_waits(nc):
    """Walrus accepts a single sync wait per instruction on this toolchain;
    split the tile epilogue drain's N waits across a chain of drains.
    (HW path only -- CoreSim's race detector rejects injected instructions.)"""
    import concourse.mybir as mybir
    nsplit = 0
    for bb in nc.main_func.blocks:
        insts = bb.instructions
        i = 0
        while i < len(insts):
            ins = insts[i]
            si = ins.sync_info
            if si is not None and si.on_wait and len(si.on_wait) > 1 \
                    and type(ins).__name__ == "InstDrain":
                waits = list(si.on_wait)
                for w in waits[:-1]:
                    nd = mybir.InstDrain(name=f"I-dsplit{nsplit}", ins=[], outs=[])
                    nsplit += 1
                    nd.engine = ins.engine
                    nd.sync_info = mybir.SyncInfo(on_wait=[w], on_update=[])
                    insts.insert(i, nd)
                    i += 1
                ins.sync_info = mybir.SyncInfo(on_wait=[waits[-1]],
                                               on_update=list(si.on_update))
            i += 1
    return nc


def _host_prep(self_state, other_state, difference, conv_w, conv_b,
               fc1_w, fc1_b, fc2_w, fc2_b, key_w, key_b, sel_w, sel_b, gumbel):
    """Build the per-core input maps (shard + layout + weight folding)."""
    Wf, bf = _fold_conv_fc1(conv_w, conv_b, fc1_w, fc1_b)

    wf_d = np.zeros((KCH, 128, 32), np.float32)
    wf_d.reshape(KF, 32)[:IN_FEAT] = Wf.T

    fc2ts = np.tile(np.asarray(fc2_w, np.float32).T, (4, 1)).astype(BF16)
    A = key_w.T @ sel_w
    aext128 = np.zeros((128, 32), np.float32)
    aext128[:32] = A / SCALE
    aext128 = aext128.astype(BF16)
    u = (sel_w.T @ key_b) / SCALE
    b1s = np.tile(bf, 4).astype(np.float32)
    b2s = np.tile(np.asarray(fc2_b, np.float32), 4)
    us = np.tile(u.astype(np.float32), 4)

    # smat4: per stack-column 0/1 selection, rows 32s+k -> out m iff m%7==a(slot)
    smat4 = np.zeros((128, 4 * 28), np.float32)
    m = np.arange(28)
    for cl in range(2):
        for s, content in enumerate(COLS[cl]):
            if content == 'self':
                continue
            for c in (cl, cl + 2):
                smat4[32 * s:32 * s + 32, 28 * c:28 * c + 28] = \
                    (m % 7 == content).astype(np.float32)[None, :]
    smat4 = smat4.astype(BF16)
    bones128 = np.zeros((128, 28), np.float32)
    bones128[:28] = (m[:, None] // 7 == m[None, :] // 7).astype(np.float32)
    hmean128 = np.zeros((128, 7), np.float32)
    hmean128[:28] = 0.25 * (m[:, None] % 7 == np.arange(7)[None, :]).astype(np.float32)
    ones128 = np.zeros((128, 1), np.float32)
    ones128[:7] = 1.0

    def blob_of(g_c, d_c):
        cb = np.zeros((128, 2240), np.float32)
        cb[:28, 0:1024] = g_c
        cb[:7, 1024:2048] = d_c
        cb[:, 2048:2076] = bones128[:, :28] * 0
        cb[:28, 2048:2076] = bones128[:28]
        cb[:28, 2076:2083] = hmean128[:28]
        cb[:7, 2083:2084] = ones128[:7]
        cb[:, 2084] = b1s
        cb[:, 2085] = b2s
        cb[:, 2086] = us
        cb[:, 2088:2104].view(BF16)[:, :32] = fc2ts
        cb[:, 2104:2160].view(BF16)[:, :112] = smat4
        cb[:32, 2160:2176].view(BF16)[:32, :32] = aext128[:32]
        cb[:, 2176:2240].view(F8)[:, :256] = \
            wf_d.transpose(1, 0, 2).reshape(128, 256)
        return cb

    xs = np.asarray(self_state, np.float32).reshape(B, T, IN_FEAT)
    xo = np.asarray(other_state, np.float32).reshape(B, T, A1, IN_FEAT)
    g = np.asarray(gumbel, np.float32)
    d = np.asarray(difference, np.float32)

    in_maps = []
    for cid in range(NCORES):
        b0, b1 = cid * BPC, (cid + 1) * BPC
        xs_c = xs[b0:b1].reshape(BT, IN_FEAT)
        xo_c = np.moveaxis(xo[b0:b1].reshape(BT, A1, IN_FEAT), 1, 0)  # [A1, BT, F]
        Xc = np.zeros((NS, KF), np.float32)
        for h in range(2):
            base, bts = h * 4096, slice(h * 512, h * 512 + 512)
            Xc[base:base + 512, :IN_FEAT] = xs_c[bts]
            for a in range(A1):
                Xc[base + 512 * (1 + a):base + 512 * (2 + a), :IN_FEAT] = xo_c[a, bts]
        xt_c = np.ascontiguousarray(Xc.T.astype(F8)).reshape(KCH, 128, NS)
        g_c = np.ascontiguousarray(
            g[:, b0:b1].reshape(HEADS, BT, A1).transpose(0, 2, 1)).reshape(28, BT)
        d_c = np.ascontiguousarray(d[b0:b1].reshape(BT, A1).T)
        in_maps.append({"xt": xt_c, "cblob": blob_of(g_c, d_c)})
    return in_maps


def _assemble(results):
    threat = np.empty((B, T, A1), np.float32)
    reward = np.empty((B, T, 1), np.float32)
    for cid, res in enumerate(results):
        b0, b1 = cid * BPC, (cid + 1) * BPC
        oc = np.asarray(res["outc"], np.float32)
        threat[b0:b1] = oc[:7].T.reshape(BPC, T, A1)
        reward[b0:b1] = oc[7].reshape(BPC, T, 1)
    return reward, threat


def kernel(self_state, other_state, difference, conv_w, conv_b,
           fc1_w, fc1_b, fc2_w, fc2_b, key_w, key_b, sel_w, sel_b, gumbel):
    from concourse.bass_utils import run_bass_kernel_spmd
    in_maps = _host_prep(self_state, other_state, difference, conv_w, conv_b,
                         fc1_w, fc1_b, fc2_w, fc2_b, key_w, key_b, sel_w, sel_b,
                         gumbel)
    cb0 = in_maps[0]["cblob"]
    key = (not cb0[:, 2084].any(), not cb0[:, 2085].any(), not cb0[:, 2086].any())
    if key not in _NC_CACHE:
        _NC_CACHE[key] = _split_drain_waits(_build_nc(*key))
    res = run_bass_kernel_spmd(_NC_CACHE[key], in_maps, list(range(NCORES)))
    return _assemble(res.results)
